# revision 1
# baseline (speedup 1.0000x reference)
"""Sparse masked attention layer for Trainium2, sharded over 8 NeuronCores.

Strategy
--------
The reference masks attention columns (keys) not in ``mask_ind`` with -inf
before softmax and zeroes rows (queries) not in ``mask_ind`` after softmax.
Both facts mean only the ~63% of token positions present in ``mask_ind``
participate at all: rows absent from the set produce exactly ``bproj`` in
the output.  So the host compacts each batch down to its kept token set,
the device runs *dense* attention on the compacted tokens (exactly equal
to the reference's masked softmax), and the host scatters results back,
filling non-kept rows with ``bproj``.

Sharding: core = (batch b, head-group g) -> 4 batches x 2 groups of 8
heads.  Each core computes q/k/v projections for its 8 heads from the
(replicated per-batch) compacted activations, attention per head, and its
partial contribution to the output projection (rows g*512:(g+1)*512 of
Wproj).  The two partials of a batch are summed on the host (D-sharded
matmul reduce) and bproj is added there.

Device layouts (per core, Cp = padded kept-token count):
  xT   [D, Cp]   compacted activations, transposed (host-side transpose)
  qkT  [128, 8, Cp] sbuf: chunks 0-3 = q features (512), 4-7 = k features
  v    [128, NC, 8, 65] sbuf: per c-chunk, per head: 64 v-features plus a
       "keep" column (1.0 for real tokens, 0.0 for padding).  The keep
       column makes the attention matmul compute the softmax denominator
       for free (row 64 of the AV output), with padded slots excluded.
  S^T  per head: psum [128 kept-k, q] = kT^T @ qT (K=64); exp via ACT with
       scale=1/8 fused.  Layout is transposed so P^T feeds the AV matmul
       directly as the moving operand (no transposes anywhere).
  attnT [64, 8, Cp] normalized attention output, transposed - exactly the
       lhsT layout the output projection needs.

All matmuls run in float32r (full-rate PE, ~1e-4 relative accuracy).
"""

import math

import numpy as np

B, C, D, H = 4, 2048, 1024, 16
HD = D // H          # 64
HPC = H // 2         # 8 heads per core
FQ = HPC * HD        # 512 per-core q/k/v feature count
N_CORES = 8

_NC_CACHE = {}


def _chunks(total, step):
    return [(i, min(step, total - i)) for i in range(0, total, step)]


def _build_nc(Cp):
    import concourse.mybir as mybir
    import concourse.tile as tile
    from concourse import bacc

    f32 = mybir.dt.float32
    f32r = mybir.dt.float32r
    Exp = mybir.ActivationFunctionType.Exp
    Ln = mybir.ActivationFunctionType.Ln

    NC = Cp // 128       # kept-token chunks of 128
    KD = D // 128        # 8 contraction chunks for the projections
    n512 = _chunks(Cp, 512)
    # q-dimension groups for attention: 512 wide (1 PSUM bank each)
    qgroups = _chunks(Cp, 512)
    qg_max = max(sz for _, sz in qgroups)

    nc = bacc.Bacc()
    xT = nc.dram_tensor("xT", [D, Cp], f32r, kind="ExternalInput")
    wqk = nc.dram_tensor("wqk", [D, 2 * FQ], f32r, kind="ExternalInput")
    bqk = nc.dram_tensor("bqk", [1, 2 * FQ], f32r, kind="ExternalInput")
    wv = nc.dram_tensor("wv", [D, FQ], f32r, kind="ExternalInput")
    bv = nc.dram_tensor("bv", [1, FQ], f32r, kind="ExternalInput")
    wp = nc.dram_tensor("wp", [FQ, D], f32r, kind="ExternalInput")
    keep = nc.dram_tensor("keep", [128, NC], f32, kind="ExternalInput")
    keepr = nc.dram_tensor("keepr", [128, NC], f32r, kind="ExternalInput")
    onesd = nc.dram_tensor("ones", [1, Cp], f32r, kind="ExternalInput")
    onesf = nc.dram_tensor("onesf", [1, 64], f32, kind="ExternalInput")
    outT = nc.dram_tensor("outT", [D, Cp], f32, kind="ExternalOutput")

    with tile.TileContext(nc) as tc:
        with tc.tile_pool(name="qkv", bufs=1) as p_qkv:
            qkT = p_qkv.tile([128, 8, Cp], f32r)
            vsb = p_qkv.tile([128, NC, HPC, HD + 1], f32r)

            # ---------------- phase A: projections ----------------
            with (
                tc.tile_pool(name="inp", bufs=1) as p_in,
                tc.tile_pool(name="psA", bufs=3, space="PSUM") as psA,
            ):
                xTs = p_in.tile([128, KD, Cp], f32r)
                wqks = p_in.tile([128, KD, 2 * FQ], f32r)
                wvs = p_in.tile([128, KD, FQ], f32r)
                for k in range(KD):
                    nc.sync.dma_start(wqks[:, k], wqk[k * 128:(k + 1) * 128, :])
                    nc.sync.dma_start(xTs[:, k], xT[k * 128:(k + 1) * 128, :])
                    nc.sync.dma_start(wvs[:, k], wv[k * 128:(k + 1) * 128, :])
                bqks = p_in.tile([1, 2 * FQ], f32r)
                nc.sync.dma_start(bqks[:], bqk[:])
                bvs = p_in.tile([1, FQ], f32r)
                nc.sync.dma_start(bvs[:], bv[:])
                keeps = p_in.tile([128, NC], f32)
                nc.sync.dma_start(keeps[:], keep[:])
                keeprs = p_in.tile([128, NC], f32r)
                nc.sync.dma_start(keeprs[:], keepr[:])
                ones = p_in.tile([1, Cp], f32r)
                nc.sync.dma_start(ones[:], onesd[:])

                # qkT[f, c] = (x @ Wqk + bqk)^T ; K=1 tail matmul adds the bias
                for m in range(8):
                    for n0, nsz in n512:
                        ps = psA.tile([128, 512], f32, tag="psA")
                        for k in range(KD):
                            nc.tensor.matmul(
                                ps[:, :nsz],
                                wqks[:, k, m * 128:(m + 1) * 128],
                                xTs[:, k, n0:n0 + nsz],
                                start=(k == 0), stop=False,
                            )
                        nc.tensor.matmul(
                            ps[:, :nsz],
                            bqks[0:1, m * 128:(m + 1) * 128],
                            ones[0:1, n0:n0 + nsz],
                            start=False, stop=True,
                        )
                        nc.vector.tensor_copy(qkT[:, m, n0:n0 + nsz], ps[:, :nsz])

                # v[c, f] = (x @ Wv + bv) * keep[c]; keep col = keep[c]
                for j in range(HPC):
                    nc.vector.tensor_copy(vsb[:, :, j, HD:HD + 1], keeprs[:])
                for c in range(NC):
                    ps = psA.tile([128, 512], f32, tag="psA")
                    for k in range(KD):
                        nc.tensor.matmul(
                            ps[:],
                            xTs[:, k, c * 128:(c + 1) * 128],
                            wvs[:, k, :],
                            start=(k == 0), stop=False,
                        )
                    nc.tensor.matmul(
                        ps[:], ones[0:1, c * 128:(c + 1) * 128], bvs[0:1, :],
                        start=False, stop=True,
                    )
                    nc.vector.tensor_scalar_mul(
                        vsb[:, c, :, 0:HD], ps[:], keeps[:, c:c + 1]
                    )

            # ---------------- phases B+C ----------------
            with (
                tc.tile_pool(name="att", bufs=2) as p_att,
                tc.tile_pool(name="pT", bufs=3) as p_pT,
                tc.tile_pool(name="attnT", bufs=1) as p_attnT,
                tc.tile_pool(name="wpp", bufs=1) as p_wp,
                tc.tile_pool(name="outs", bufs=3) as p_out,
            ):
                attnT = p_attnT.tile([128, HPC // 2, Cp], f32r)
                wps = p_wp.tile([128, HPC // 2, D], f32r)
                nc.sync.dma_start(wps[:], wp[:].rearrange("(c p) n -> p c n", p=128))
                onesfs = p_att.tile([1, 64], f32, tag="onesf", bufs=1)
                nc.sync.dma_start(onesfs[:], onesf[:])

                # phase B: attention.  Head pairs share the PE via row
                # tiling (even head in array rows 0-63, odd in 64-127).
                with (
                    tc.tile_pool(name="psS", bufs=2, space="PSUM") as psS,
                    tc.tile_pool(name="psAV", bufs=4, space="PSUM") as psAV,
                    tc.tile_pool(name="psBC", bufs=2, space="PSUM") as psBC,
                ):
                    for hp in range(4):
                        heads = (2 * hp, 2 * hp + 1)
                        for q0, qsz in qgroups:
                            avs = []
                            for hi, h in enumerate(heads):
                                avs.append(psAV.tile([65, qg_max], f32, tag="av",
                                                     name=f"av_{hp}_{q0}_{hi}"))
                            for kc in range(NC):
                                sss, pTs = [], []
                                for hi, h in enumerate(heads):
                                    lo = hi * 64
                                    ss = psS.tile([128, qg_max], f32, tag="ss")
                                    for s0, ssz in _chunks(qsz, 512):
                                        nc.tensor.matmul(
                                            ss[:, s0:s0 + ssz],
                                            qkT[lo:lo + 64, 4 + hp, kc * 128:(kc + 1) * 128],
                                            qkT[lo:lo + 64, hp, q0 + s0:q0 + s0 + ssz],
                                            start=True, stop=True,
                                        )
                                    sss.append(ss)
                                for hi, h in enumerate(heads):
                                    pT = p_pT.tile([128, qg_max], f32r, tag="pT")
                                    nc.scalar.activation(
                                        pT[:, :qsz], sss[hi][:, :qsz], Exp, scale=0.125
                                    )
                                    pTs.append(pT)
                                for hi, h in enumerate(heads):
                                    for s0, ssz in _chunks(qsz, 512):
                                        nc.tensor.matmul(
                                            avs[hi][:, s0:s0 + ssz],
                                            vsb[:, kc, h, :],
                                            pTs[hi][:, s0:s0 + ssz],
                                            start=(kc == 0), stop=(kc == NC - 1),
                                        )
                            for hi, h in enumerate(heads):
                                av = avs[hi]
                                # 1/denom on DVE (fast approx, ~18 bits), then
                                # broadcast across partitions via a K=1 PE
                                # outer product with a ones column.
                                dcp = p_att.tile([1, qg_max], f32, tag="dcp")
                                nc.vector.tensor_copy(dcp[0:1, :qsz],
                                                      av[64:65, :qsz])
                                rec = p_att.tile([1, qg_max], f32, tag="rec")
                                nc.vector.reciprocal_approx_fast(
                                    rec[0:1, :qsz], dcp[0:1, :qsz])
                                bcp = psBC.tile([64, qg_max], f32, tag="bc",
                                                name=f"bc_{hp}_{q0}_{hi}")
                                nc.tensor.matmul(bcp[:, :qsz], onesfs[0:1, :],
                                                 rec[0:1, :qsz],
                                                 start=True, stop=True)
                                bcs = p_att.tile([64, qg_max], f32, tag="bcs")
                                nc.scalar.copy(bcs[:, :qsz], bcp[:, :qsz])
                                lo = (h % 2) * 64
                                nc.vector.tensor_mul(
                                    attnT[lo:lo + 64, h // 2, q0:q0 + qsz],
                                    av[0:64, :qsz],
                                    bcs[:, :qsz],
                                )

                # phase C: output projection partial, transposed out
                with tc.tile_pool(name="psC", bufs=2, space="PSUM") as psC:
                    for m in range(8):
                        for n0, nsz in n512:
                            ps = psC.tile([128, 512], f32, tag="psC")
                            for j in range(HPC // 2):
                                nc.tensor.matmul(
                                    ps[:, :nsz],
                                    wps[:, j, m * 128:(m + 1) * 128],
                                    attnT[:, j, n0:n0 + nsz],
                                    start=(j == 0), stop=(j == HPC // 2 - 1),
                                )
                            st = p_out.tile([128, 512], f32, tag="st")
                            nc.vector.tensor_copy(st[:, :nsz], ps[:, :nsz])
                            nc.sync.dma_start(
                                outT[m * 128:(m + 1) * 128, n0:n0 + nsz], st[:, :nsz]
                            )

    nc.finalize()
    return nc


def _get_nc(Cp):
    if Cp not in _NC_CACHE:
        _NC_CACHE[Cp] = _build_nc(Cp)
    return _NC_CACHE[Cp]


def kernel(x, mask_ind, Wqkv, bqkv, Wproj, bproj, **_unused):
    from concourse.bass_utils import run_bass_kernel_spmd

    x = np.asarray(x, dtype=np.float32)
    mask_ind = np.asarray(mask_ind)
    Wqkv = np.asarray(Wqkv, dtype=np.float32)
    bqkv = np.asarray(bqkv, dtype=np.float32)
    Wproj = np.asarray(Wproj, dtype=np.float32)
    bproj = np.asarray(bproj, dtype=np.float32)

    # kept-token sets per batch (matches reference _keep_mask semantics)
    idx = []
    for b in range(B):
        mi = mask_ind[b]
        mi = mi[mi >= 0]
        mi = np.clip(mi, 0, C - 1)
        idx.append(np.unique(mi).astype(np.int64))
    nmax = max(len(u) for u in idx)
    Cp = max(128, ((nmax + 127) // 128) * 128)
    NC = Cp // 128

    nc = _get_nc(Cp)

    in_maps = []
    for core in range(N_CORES):
        b, g = core // 2, core % 2
        u = idx[b]
        n = len(u)
        xk = np.zeros((Cp, D), dtype=np.float32)
        xk[:n] = x[b, u]
        keep = np.zeros(Cp, dtype=np.float32)
        keep[:n] = 1.0
        qs, ks, vs = g * FQ, D + g * FQ, 2 * D + g * FQ
        wqk = np.concatenate(
            [Wqkv[:, qs:qs + FQ], Wqkv[:, ks:ks + FQ]], axis=1
        )
        bqk = np.concatenate([bqkv[qs:qs + FQ], bqkv[ks:ks + FQ]])
        in_maps.append({
            "xT": np.ascontiguousarray(xk.T),
            "wqk": np.ascontiguousarray(wqk),
            "bqk": bqk.reshape(1, -1),
            "wv": np.ascontiguousarray(Wqkv[:, vs:vs + FQ]),
            "bv": bqkv[vs:vs + FQ].reshape(1, -1).copy(),
            "wp": np.ascontiguousarray(Wproj[g * FQ:(g + 1) * FQ, :]),
            "keep": np.ascontiguousarray(keep.reshape(NC, 128).T),
            "keepr": np.ascontiguousarray(keep.reshape(NC, 128).T),
            "ones": np.ones((1, Cp), dtype=np.float32),
            "onesf": np.ones((1, 64), dtype=np.float32),
        })

    global _last_in_maps
    _last_in_maps = in_maps
    res = run_bass_kernel_spmd(nc, in_maps, core_ids=list(range(N_CORES)))

    out = np.broadcast_to(bproj, (B, C, D)).copy()
    for b in range(B):
        u = idx[b]
        n = len(u)
        comb = res.results[2 * b]["outT"] + res.results[2 * b + 1]["outT"]
        out[b, u] += comb.T[:n]
    return out



# revision 11
# speedup vs baseline: 1.7305x; 1.7305x over previous
"""Sparse masked attention layer for Trainium2, sharded over 8 NeuronCores.

Strategy
--------
The reference masks attention columns (keys) not in ``mask_ind`` with -inf
before softmax and zeroes rows (queries) not in ``mask_ind`` after softmax.
Both facts mean only the ~63% of token positions present in ``mask_ind``
participate at all: rows absent from the set produce exactly ``bproj`` in
the output.  So the host compacts each batch down to its kept token set,
the device runs *dense* attention on the compacted tokens (exactly equal
to the reference's masked softmax), and the host scatters results back,
filling non-kept rows with ``bproj``.

Sharding: core = (batch b, head-group g) -> 4 batches x 2 groups of 8
heads.  Each core computes q/k/v projections for its 8 heads from the
(replicated per-batch) compacted activations, attention per head, and its
partial contribution to the output projection (rows g*512:(g+1)*512 of
Wproj).  The two partials of a batch are summed on the host (D-sharded
matmul reduce) and bproj is added there.

All matmuls run in bfloat16 (fp32 PSUM accumulate).  fp32(r) matmuls
measured ~2.5-4.3 cycles/row on hardware; bf16 streams at the full PE
rate and halves LDWEIGHTS + SBUF traffic.  The softmax denominator keeps
its exactness: the keep-column trick sums the *same* bf16 pT values the
AV matmul consumes, so normalization cancels pT rounding.

Device layouts (per core, Cp = padded kept-token count):
  xT   [D, Cp]   compacted activations, transposed (host-side transpose)
  qkT  [128, 8, Cp] sbuf bf16: chunks 0-3 = q features, 4-7 = k features
  v    [128, NC, 8, 65] sbuf bf16: per c-chunk, per head: 64 v-features
       plus a "keep" column (1.0 real token, 0.0 padding) that makes the
       AV matmul emit the softmax denominator for free (row 64).
  S    pairs: one [128, 2, 512] PSUM tile holds both heads of a pair, so
       a single ACT instruction exponentiates 2*qsz elements.
  attnT [64, 8, Cp] normalized attention output, transposed - exactly the
       lhsT layout the output projection needs.
"""

import numpy as np
import ml_dtypes

BF16 = ml_dtypes.bfloat16

B, C, D, H = 4, 2048, 1024, 16
HD = D // H          # 64
HPC = H // 2         # 8 heads per core
FQ = HPC * HD        # 512 per-core q/k/v feature count
N_CORES = 8

_NC_CACHE = {}


def _chunks(total, step):
    return [(i, min(step, total - i)) for i in range(0, total, step)]


def _build_nc(Cp, with_bias):
    import concourse.mybir as mybir
    import concourse.tile as tile
    from concourse import bacc

    f32 = mybir.dt.float32
    f32r = mybir.dt.float32r
    bf16 = mybir.dt.bfloat16
    Exp = mybir.ActivationFunctionType.Exp

    NC = Cp // 128       # kept-token chunks of 128
    KD = D // 128        # 8 contraction chunks for the projections
    n512 = _chunks(Cp, 512)
    qgroups = _chunks(Cp, 512)

    nc = bacc.Bacc()
    xT = nc.dram_tensor("xT", [D, Cp], bf16, kind="ExternalInput")
    wqk = nc.dram_tensor("wqk", [D, 2 * FQ], bf16, kind="ExternalInput")
    wv = nc.dram_tensor("wv", [D, FQ], bf16, kind="ExternalInput")
    wp = nc.dram_tensor("wp", [FQ, D], bf16, kind="ExternalInput")
    keep = nc.dram_tensor("keep", [128, NC], f32, kind="ExternalInput")
    keepb = nc.dram_tensor("keepb", [128, NC], bf16, kind="ExternalInput")
    onesf = nc.dram_tensor("onesf", [1, 64], bf16, kind="ExternalInput")
    if with_bias:
        bqk = nc.dram_tensor("bqk", [1, 2 * FQ], bf16, kind="ExternalInput")
        bv = nc.dram_tensor("bv", [1, FQ], bf16, kind="ExternalInput")
        onesd = nc.dram_tensor("ones", [1, Cp], bf16, kind="ExternalInput")
    outT = nc.dram_tensor("outT", [D, Cp], f32, kind="ExternalOutput")

    with tile.TileContext(nc) as tc:
        with tc.tile_pool(name="qkv", bufs=1) as p_qkv:
            qkT = p_qkv.tile([128, 8, Cp], bf16)
            vsb = p_qkv.tile([128, NC, HPC, HD + 2], bf16)

            # ---------------- phase A: projections ----------------
            with (
                tc.tile_pool(name="inp", bufs=1) as p_in,
                tc.tile_pool(name="psA", bufs=3, space="PSUM") as psA,
            ):
                xTs = p_in.tile([128, KD, Cp], bf16)
                wqks = p_in.tile([128, KD, 2 * FQ], bf16)
                wvs = p_in.tile([128, KD, FQ], bf16)
                for k in range(KD):
                    nc.sync.dma_start(wqks[:, k], wqk[k * 128:(k + 1) * 128, :])
                    nc.sync.dma_start(xTs[:, k], xT[k * 128:(k + 1) * 128, :])
                    nc.sync.dma_start(wvs[:, k], wv[k * 128:(k + 1) * 128, :])
                keeps = p_in.tile([128, NC], f32)
                nc.sync.dma_start(keeps[:], keep[:])
                keepbs = p_in.tile([128, NC], bf16)
                nc.sync.dma_start(keepbs[:], keepb[:])
                if with_bias:
                    bqks = p_in.tile([1, 2 * FQ], bf16)
                    nc.sync.dma_start(bqks[:], bqk[:])
                    bvs = p_in.tile([1, FQ], bf16)
                    nc.sync.dma_start(bvs[:], bv[:])
                    ones = p_in.tile([1, Cp], bf16)
                    nc.sync.dma_start(ones[:], onesd[:])

                # qkT[f, c] = (x @ Wqk + bqk)^T
                for m in range(8):
                    for n0, nsz in n512:
                        ps = psA.tile([128, 512], f32, tag="psA")
                        for k in range(KD):
                            nc.tensor.matmul(
                                ps[:, :nsz],
                                wqks[:, k, m * 128:(m + 1) * 128],
                                xTs[:, k, n0:n0 + nsz],
                                start=(k == 0), stop=(k == KD - 1) and not with_bias,
                            )
                        if with_bias:
                            nc.tensor.matmul(
                                ps[:, :nsz],
                                bqks[0:1, m * 128:(m + 1) * 128],
                                ones[0:1, n0:n0 + nsz],
                                start=False, stop=True,
                            )
                        nc.vector.tensor_copy(qkT[:, m, n0:n0 + nsz], ps[:, :nsz])

                # v[c, f] = (x @ Wv + bv) * keep[c]; keep col = keep[c]
                for j in range(HPC):
                    nc.vector.tensor_copy(vsb[:, :, j, HD:HD + 1], keepbs[:])
                    nc.vector.memset(vsb[:, :, j, HD + 1:HD + 2], 0)
                for c in range(NC):
                    ps = psA.tile([128, 512], f32, tag="psA")
                    for k in range(KD):
                        nc.tensor.matmul(
                            ps[:],
                            xTs[:, k, c * 128:(c + 1) * 128],
                            wvs[:, k, :],
                            start=(k == 0), stop=(k == KD - 1) and not with_bias,
                        )
                    if with_bias:
                        nc.tensor.matmul(
                            ps[:], ones[0:1, c * 128:(c + 1) * 128], bvs[0:1, :],
                            start=False, stop=True,
                        )
                    nc.vector.tensor_scalar_mul(
                        vsb[:, c, :, 0:HD], ps[:], keeps[:, c:c + 1]
                    )

            # ---------------- phase B: attention ----------------
            with (
                tc.tile_pool(name="att", bufs=2) as p_att,
                tc.tile_pool(name="pT", bufs=3) as p_pT,
                tc.tile_pool(name="attnT", bufs=1) as p_attnT,
                tc.tile_pool(name="wpp", bufs=1) as p_wp,
            ):
                attnT = p_attnT.tile([128, HPC // 2, Cp], bf16)
                wps = p_wp.tile([128, HPC // 2, D], bf16)
                nc.sync.dma_start(wps[:], wp[:].rearrange("(c p) n -> p c n", p=128))
                onesfs = p_att.tile([1, 64], bf16, tag="onesf", bufs=1)
                nc.sync.dma_start(onesfs[:], onesf[:])

                with (
                    tc.tile_pool(name="psS", bufs=2, space="PSUM") as psS,
                    tc.tile_pool(name="psAV", bufs=3, space="PSUM") as psAV,
                    tc.tile_pool(name="psBC", bufs=1, space="PSUM") as psBC,
                ):
                    for q0, qsz in qgroups:
                        for hp in range(4):
                            heads = (2 * hp, 2 * hp + 1)
                            avs = [
                                psAV.tile([66, 512], f32, tag="av",
                                          name=f"av_{hp}_{q0}_{hi}")
                                for hi in range(2)
                            ]
                            # software-pipelined S -> exp -> AV: emit S(kc)
                            # then AV(kc-1) so the PE never sits behind an
                            # un-exponentiated AV in its queue.
                            prev_pT = None
                            for kc in range(NC):
                                ss = psS.tile([128, 2, 512], f32, tag="ss")
                                for hi, h in enumerate(heads):
                                    lo = hi * 64
                                    nc.tensor.matmul(
                                        ss[:, hi, :qsz],
                                        qkT[lo:lo + 64, 4 + hp, kc * 128:(kc + 1) * 128],
                                        qkT[lo:lo + 64, hp, q0:q0 + qsz],
                                        start=True, stop=True,
                                    )
                                pT = p_pT.tile([128, 2, 512], bf16, tag="pT")
                                for hi in range(2):
                                    nc.scalar.activation(
                                        pT[:, hi, :qsz], ss[:, hi, :qsz], Exp,
                                        scale=0.125
                                    )
                                if prev_pT is not None:
                                    pkc, ppT = prev_pT
                                    for hi, h in enumerate(heads):
                                        nc.tensor.matmul(
                                            avs[hi][:, :qsz],
                                            vsb[:, pkc, h, :],
                                            ppT[:, hi, :qsz],
                                            start=(pkc == 0), stop=False,
                                        )
                                prev_pT = (kc, pT)
                            pkc, ppT = prev_pT
                            for hi, h in enumerate(heads):
                                nc.tensor.matmul(
                                    avs[hi][:, :qsz],
                                    vsb[:, pkc, h, :],
                                    ppT[:, hi, :qsz],
                                    start=(pkc == 0), stop=True,
                                )
                            for hi, h in enumerate(heads):
                                av = avs[hi]
                                # 1/denom on DVE, broadcast across partitions
                                # via a K=1 PE outer product (f32r: keeps the
                                # denominator fp32-exact).
                                dcp = p_att.tile([1, 512], f32, tag="dcp")
                                nc.vector.tensor_copy(dcp[0:1, :qsz],
                                                      av[64:65, :qsz])
                                rec = p_att.tile([1, 512], f32, tag="rec")
                                nc.vector.reciprocal_approx_fast(
                                    rec[0:1, :qsz], dcp[0:1, :qsz])
                                recb = p_att.tile([1, 512], bf16, tag="recb")
                                nc.vector.tensor_copy(recb[0:1, :qsz],
                                                      rec[0:1, :qsz])
                                bcp = psBC.tile([64, 512], f32, tag="bc",
                                                name=f"bc_{hp}_{q0}_{hi}")
                                nc.tensor.matmul(
                                    bcp[:, :qsz], onesfs[0:1, :],
                                    recb[0:1, :qsz],
                                    start=True, stop=True)
                                bcs = p_att.tile([64, 512], f32, tag="bcs")
                                nc.vector.tensor_copy(bcs[:, :qsz], bcp[:, :qsz])
                                lo = hi * 64
                                nc.vector.tensor_mul(
                                    attnT[lo:lo + 64, hp, q0:q0 + qsz],
                                    av[0:64, :qsz],
                                    bcs[:, :qsz],
                                )

                # phase C: output projection partial, transposed out
                with (
                    tc.tile_pool(name="psC", bufs=3, space="PSUM") as psC,
                    tc.tile_pool(name="outs", bufs=3) as p_out,
                ):
                    for m in range(8):
                        for n0, nsz in n512:
                            ps = psC.tile([128, 512], f32, tag="psC")
                            for j in range(HPC // 2):
                                nc.tensor.matmul(
                                    ps[:, :nsz],
                                    wps[:, j, m * 128:(m + 1) * 128],
                                    attnT[:, j, n0:n0 + nsz],
                                    start=(j == 0), stop=(j == HPC // 2 - 1),
                                )
                            st = p_out.tile([128, 512], f32, tag="st")
                            nc.vector.tensor_copy(st[:, :nsz], ps[:, :nsz])
                            nc.sync.dma_start(
                                outT[m * 128:(m + 1) * 128, n0:n0 + nsz], st[:, :nsz]
                            )

    nc.finalize()
    return nc


def _get_nc(Cp, with_bias):
    key = (Cp, with_bias)
    if key not in _NC_CACHE:
        _NC_CACHE[key] = _build_nc(Cp, with_bias)
    return _NC_CACHE[key]


def kernel(x, mask_ind, Wqkv, bqkv, Wproj, bproj, **_unused):
    from concourse.bass_utils import run_bass_kernel_spmd

    x = np.asarray(x, dtype=np.float32)
    mask_ind = np.asarray(mask_ind)
    Wqkv = np.asarray(Wqkv, dtype=np.float32)
    bqkv = np.asarray(bqkv, dtype=np.float32)
    Wproj = np.asarray(Wproj, dtype=np.float32)
    bproj = np.asarray(bproj, dtype=np.float32)

    # kept-token sets per batch (matches reference _keep_mask semantics)
    idx = []
    for b in range(B):
        mi = mask_ind[b]
        mi = mi[mi >= 0]
        mi = np.clip(mi, 0, C - 1)
        idx.append(np.unique(mi).astype(np.int64))
    nmax = max(len(u) for u in idx)
    Cp = max(128, ((nmax + 127) // 128) * 128)
    NC = Cp // 128

    with_bias = bool(np.any(bqkv != 0.0))
    nc = _get_nc(Cp, with_bias)

    in_maps = []
    for core in range(N_CORES):
        b, g = core // 2, core % 2
        u = idx[b]
        n = len(u)
        xk = np.zeros((Cp, D), dtype=np.float32)
        xk[:n] = x[b, u]
        keep = np.zeros(Cp, dtype=np.float32)
        keep[:n] = 1.0
        qs, ks, vs = g * FQ, D + g * FQ, 2 * D + g * FQ
        wqk = np.concatenate(
            [Wqkv[:, qs:qs + FQ], Wqkv[:, ks:ks + FQ]], axis=1
        )
        im = {
            "xT": np.ascontiguousarray(xk.T).astype(BF16),
            "wqk": np.ascontiguousarray(wqk).astype(BF16),
            "wv": np.ascontiguousarray(Wqkv[:, vs:vs + FQ]).astype(BF16),
            "wp": np.ascontiguousarray(Wproj[g * FQ:(g + 1) * FQ, :]).astype(BF16),
            "keep": np.ascontiguousarray(keep.reshape(NC, 128).T),
            "keepb": np.ascontiguousarray(keep.reshape(NC, 128).T).astype(BF16),
            "onesf": np.ones((1, 64), dtype=BF16),
        }
        if with_bias:
            bqk = np.concatenate([bqkv[qs:qs + FQ], bqkv[ks:ks + FQ]])
            im["bqk"] = bqk.reshape(1, -1).astype(BF16)
            im["bv"] = bqkv[vs:vs + FQ].reshape(1, -1).astype(BF16)
            im["ones"] = np.ones((1, Cp), dtype=BF16)
        in_maps.append(im)

    global _last_in_maps
    _last_in_maps = in_maps
    res = run_bass_kernel_spmd(nc, in_maps, core_ids=list(range(N_CORES)))

    out = np.broadcast_to(bproj, (B, C, D)).copy()
    for b in range(B):
        u = idx[b]
        n = len(u)
        comb = res.results[2 * b]["outT"] + res.results[2 * b + 1]["outT"]
        out[b, u] += comb.T[:n]
    return out


# revision 13
# speedup vs baseline: 2.2278x; 1.2874x over previous
"""Sparse masked attention layer for Trainium2, sharded over 8 NeuronCores.

Strategy
--------
The reference masks attention columns (keys) not in ``mask_ind`` with -inf
before softmax and zeroes rows (queries) not in ``mask_ind`` after softmax.
Both facts mean only the ~63% of token positions present in ``mask_ind``
participate at all: rows absent from the set produce exactly ``bproj`` in
the output.  So the host compacts each batch down to its kept token set,
the device runs *dense* attention on the compacted tokens (exactly equal
to the reference's masked softmax), and the host scatters results back,
filling non-kept rows with ``bproj``.

Sharding: core = (batch b, head-group g) -> 4 batches x 2 groups of 8
heads.  Each core computes q/k/v projections for its 8 heads from the
(replicated per-batch) compacted activations, attention per head, and its
partial contribution to the output projection (rows g*512:(g+1)*512 of
Wproj).  The two partials of a batch are summed on the host (D-sharded
matmul reduce) and bproj is added there.

All matmuls run in bfloat16 (fp32 PSUM accumulate); fp32(r) matmuls
measured 2.5-4.3 cycles/row on hardware while bf16 streams at ~1.
The softmax denominator stays exact w.r.t. pT rounding: the keep-column
trick sums the *same* bf16 pT values the AV matmul consumes.

Schedule notes (what the trace drove):
- The PE DVFS ramp (0.65 -> 1.2 -> 2.4 GHz after ~3us continuous busy)
  punishes every stall, so the PE queue must never wait: deep PSUM
  pipelining in the projections (psA bufs=6), S->exp->AV software
  pipelined with lookahead 2, and the softmax normalization runs
  entirely off-PE (DVE reciprocal + GpSimd partition_broadcast + DVE
  multiply) so no matmul ever sits behind the DVE chain.
- One ACT exp instruction covers a head *pair* ([128, 2, qsz] strided
  over two PSUM banks) halving ACT instruction overhead.

Device layouts (per core, Cp = padded kept-token count):
  xT   [D, Cp]   compacted activations, transposed (host-side transpose)
  qkT  [128, 8, Cp] sbuf bf16: chunks 0-3 = q features, 4-7 = k features
  v    [128, NC, 8, 66] sbuf bf16: per c-chunk, per head: 64 v-features,
       a "keep" column (1.0 real token, 0.0 padding) that makes the AV
       matmul emit the softmax denominator for free (row 64), and a pad.
  attnT [64, 8, Cp] normalized attention output, transposed - exactly the
       lhsT layout the output projection needs.
"""

import numpy as np
import ml_dtypes

BF16 = ml_dtypes.bfloat16

B, C, D, H = 4, 2048, 1024, 16
HD = D // H          # 64
HPC = H // 2         # 8 heads per core
FQ = HPC * HD        # 512 per-core q/k/v feature count
N_CORES = 8

_NC_CACHE = {}


def _chunks(total, step):
    return [(i, min(step, total - i)) for i in range(0, total, step)]


def _build_nc(Cp, with_bias):
    import concourse.mybir as mybir
    import concourse.tile as tile
    from concourse import bacc

    f32 = mybir.dt.float32
    bf16 = mybir.dt.bfloat16
    Exp = mybir.ActivationFunctionType.Exp

    NC = Cp // 128       # kept-token chunks of 128
    KD = D // 128        # 8 contraction chunks for the projections
    n512 = _chunks(Cp, 512)
    qgroups = _chunks(Cp, 512)

    nc = bacc.Bacc()
    xT = nc.dram_tensor("xT", [D, Cp], bf16, kind="ExternalInput")
    wqk = nc.dram_tensor("wqk", [D, 2 * FQ], bf16, kind="ExternalInput")
    wv = nc.dram_tensor("wv", [D, FQ], bf16, kind="ExternalInput")
    wp = nc.dram_tensor("wp", [FQ, D], bf16, kind="ExternalInput")
    keep = nc.dram_tensor("keep", [128, NC], f32, kind="ExternalInput")
    keepb = nc.dram_tensor("keepb", [128, NC], bf16, kind="ExternalInput")
    if with_bias:
        bqk = nc.dram_tensor("bqk", [1, 2 * FQ], bf16, kind="ExternalInput")
        bv = nc.dram_tensor("bv", [1, FQ], bf16, kind="ExternalInput")
        onesd = nc.dram_tensor("ones", [1, Cp], bf16, kind="ExternalInput")
    outT = nc.dram_tensor("outT", [D, Cp], f32, kind="ExternalOutput")

    with tile.TileContext(nc) as tc:
        with tc.tile_pool(name="qkv", bufs=1) as p_qkv:
            qkT = p_qkv.tile([128, 8, Cp], bf16)
            vsb = p_qkv.tile([128, NC, HPC, HD + 2], bf16)

            # ---------------- phase A: projections ----------------
            with (
                tc.tile_pool(name="inp", bufs=1) as p_in,
                tc.tile_pool(name="psA", bufs=6, space="PSUM") as psA,
            ):
                xTs = p_in.tile([128, KD, Cp], bf16)
                wqks = p_in.tile([128, KD, 2 * FQ], bf16)
                wvs = p_in.tile([128, KD, FQ], bf16)
                for k in range(KD):
                    nc.sync.dma_start(wqks[:, k], wqk[k * 128:(k + 1) * 128, :])
                    nc.sync.dma_start(xTs[:, k], xT[k * 128:(k + 1) * 128, :])
                    nc.sync.dma_start(wvs[:, k], wv[k * 128:(k + 1) * 128, :])
                keeps = p_in.tile([128, NC], f32)
                nc.sync.dma_start(keeps[:], keep[:])
                keepbs = p_in.tile([128, NC], bf16)
                nc.sync.dma_start(keepbs[:], keepb[:])
                if with_bias:
                    bqks = p_in.tile([1, 2 * FQ], bf16)
                    nc.sync.dma_start(bqks[:], bqk[:])
                    bvs = p_in.tile([1, FQ], bf16)
                    nc.sync.dma_start(bvs[:], bv[:])
                    ones = p_in.tile([1, Cp], bf16)
                    nc.sync.dma_start(ones[:], onesd[:])

                # qkT[f, c] = (x @ Wqk + bqk)^T
                for m in range(8):
                    for n0, nsz in n512:
                        ps = psA.tile([128, 512], f32, tag="psA")
                        for k in range(KD):
                            nc.tensor.matmul(
                                ps[:, :nsz],
                                wqks[:, k, m * 128:(m + 1) * 128],
                                xTs[:, k, n0:n0 + nsz],
                                start=(k == 0), stop=(k == KD - 1) and not with_bias,
                            )
                        if with_bias:
                            nc.tensor.matmul(
                                ps[:, :nsz],
                                bqks[0:1, m * 128:(m + 1) * 128],
                                ones[0:1, n0:n0 + nsz],
                                start=False, stop=True,
                            )
                        nc.vector.tensor_copy(qkT[:, m, n0:n0 + nsz], ps[:, :nsz])

                # v[c, f] = (x @ Wv + bv) * keep[c]; keep col = keep[c]
                for j in range(HPC):
                    nc.vector.tensor_copy(vsb[:, :, j, HD:HD + 1], keepbs[:])
                    nc.vector.memset(vsb[:, :, j, HD + 1:HD + 2], 0)
                for c in range(NC):
                    ps = psA.tile([128, 512], f32, tag="psA")
                    for k in range(KD):
                        nc.tensor.matmul(
                            ps[:],
                            xTs[:, k, c * 128:(c + 1) * 128],
                            wvs[:, k, :],
                            start=(k == 0), stop=(k == KD - 1) and not with_bias,
                        )
                    if with_bias:
                        nc.tensor.matmul(
                            ps[:], ones[0:1, c * 128:(c + 1) * 128], bvs[0:1, :],
                            start=False, stop=True,
                        )
                    nc.vector.tensor_scalar_mul(
                        vsb[:, c, :, 0:HD], ps[:], keeps[:, c:c + 1]
                    )

            # ---------------- phase B: attention ----------------
            with (
                tc.tile_pool(name="att", bufs=2) as p_att,
                tc.tile_pool(name="pT", bufs=4) as p_pT,
                tc.tile_pool(name="attnT", bufs=1) as p_attnT,
                tc.tile_pool(name="wpp", bufs=1) as p_wp,
            ):
                attnT = p_attnT.tile([128, HPC // 2, Cp], bf16)
                wps = p_wp.tile([128, HPC // 2, D], bf16)
                nc.sync.dma_start(wps[:], wp[:].rearrange("(c p) n -> p c n", p=128))

                def norm_tail(avs, hp, q0, qsz):
                    # softmax divide, entirely off the PE: denominators sit
                    # in row 64 of each AV accumulator (keep-column trick).
                    bcss = []
                    for hi in range(2):
                        av = avs[hi]
                        dcp = p_att.tile([1, 512], f32, tag=f"dcp{hi}")
                        nc.vector.tensor_copy(dcp[0:1, :qsz], av[64:65, :qsz])
                        rec = p_att.tile([1, 512], f32, tag=f"rec{hi}")
                        nc.vector.reciprocal_approx_fast(
                            rec[0:1, :qsz], dcp[0:1, :qsz])
                        recb = p_att.tile([1, 512], bf16, tag=f"recb{hi}")
                        nc.vector.tensor_copy(recb[0:1, :qsz], rec[0:1, :qsz])
                        bcs = p_att.tile([64, 512], bf16, tag=f"bcs{hi}")
                        nc.gpsimd.partition_broadcast(
                            bcs[:, :qsz], recb[0:1, :qsz])
                        bcss.append(bcs)
                    for hi in range(2):
                        nc.vector.tensor_mul(
                            attnT[hi * 64:hi * 64 + 64, hp, q0:q0 + qsz],
                            avs[hi][0:64, :qsz],
                            bcss[hi][:, :qsz],
                        )

                with (
                    tc.tile_pool(name="psS", bufs=2, space="PSUM") as psS,
                    tc.tile_pool(name="psAV", bufs=4, space="PSUM") as psAV,
                ):
                    pending = None
                    for q0, qsz in qgroups:
                        for hp in range(4):
                            heads = (2 * hp, 2 * hp + 1)
                            avs = [
                                psAV.tile([66, 512], f32, tag="av",
                                          name=f"av_{hp}_{q0}_{hi}")
                                for hi in range(2)
                            ]
                            # software pipeline with lookahead 2:
                            # S(kc)+exp(kc) run two iterations ahead of
                            # AV(kc) so the PE never waits on the ACT.
                            pTs = [None] * NC
                            for kc in range(NC):
                                ss = psS.tile([128, 2, 512], f32, tag="ss")
                                for hi, h in enumerate(heads):
                                    lo = hi * 64
                                    nc.tensor.matmul(
                                        ss[:, hi, :qsz],
                                        qkT[lo:lo + 64, 4 + hp, kc * 128:(kc + 1) * 128],
                                        qkT[lo:lo + 64, hp, q0:q0 + qsz],
                                        start=True, stop=True,
                                    )
                                pT = p_pT.tile([128, 2, 512], bf16, tag="pT")
                                nc.scalar.activation(
                                    pT[:, :, :qsz], ss[:, :, :qsz], Exp,
                                    scale=0.125
                                )
                                pTs[kc] = pT
                                if pending is not None and kc == 1:
                                    norm_tail(*pending)
                                    pending = None
                                if kc >= 2:
                                    for hi, h in enumerate(heads):
                                        nc.tensor.matmul(
                                            avs[hi][:, :qsz],
                                            vsb[:, kc - 2, h, :],
                                            pTs[kc - 2][:, hi, :qsz],
                                            start=(kc - 2 == 0), stop=False,
                                        )
                                    pTs[kc - 2] = None
                            for kc in (NC - 2, NC - 1):
                                for hi, h in enumerate(heads):
                                    nc.tensor.matmul(
                                        avs[hi][:, :qsz],
                                        vsb[:, kc, h, :],
                                        pTs[kc][:, hi, :qsz],
                                        start=(kc == 0), stop=(kc == NC - 1),
                                    )
                            pending = (avs, hp, q0, qsz)
                    norm_tail(*pending)

                # phase C: output projection partial, transposed out
                with (
                    tc.tile_pool(name="psC", bufs=3, space="PSUM") as psC,
                    tc.tile_pool(name="outs", bufs=3) as p_out,
                ):
                    for m in range(8):
                        for n0, nsz in n512:
                            ps = psC.tile([128, 512], f32, tag="psC")
                            for j in range(HPC // 2):
                                nc.tensor.matmul(
                                    ps[:, :nsz],
                                    wps[:, j, m * 128:(m + 1) * 128],
                                    attnT[:, j, n0:n0 + nsz],
                                    start=(j == 0), stop=(j == HPC // 2 - 1),
                                )
                            st = p_out.tile([128, 512], f32, tag="st")
                            nc.vector.tensor_copy(st[:, :nsz], ps[:, :nsz])
                            nc.sync.dma_start(
                                outT[m * 128:(m + 1) * 128, n0:n0 + nsz], st[:, :nsz]
                            )

    nc.finalize()
    return nc


def _get_nc(Cp, with_bias):
    key = (Cp, with_bias)
    if key not in _NC_CACHE:
        _NC_CACHE[key] = _build_nc(Cp, with_bias)
    return _NC_CACHE[key]


def kernel(x, mask_ind, Wqkv, bqkv, Wproj, bproj, **_unused):
    from concourse.bass_utils import run_bass_kernel_spmd

    x = np.asarray(x, dtype=np.float32)
    mask_ind = np.asarray(mask_ind)
    Wqkv = np.asarray(Wqkv, dtype=np.float32)
    bqkv = np.asarray(bqkv, dtype=np.float32)
    Wproj = np.asarray(Wproj, dtype=np.float32)
    bproj = np.asarray(bproj, dtype=np.float32)

    # kept-token sets per batch (matches reference _keep_mask semantics)
    idx = []
    for b in range(B):
        mi = mask_ind[b]
        mi = mi[mi >= 0]
        mi = np.clip(mi, 0, C - 1)
        idx.append(np.unique(mi).astype(np.int64))
    nmax = max(len(u) for u in idx)
    Cp = max(256, ((nmax + 127) // 128) * 128)
    NC = Cp // 128

    with_bias = bool(np.any(bqkv != 0.0))
    nc = _get_nc(Cp, with_bias)

    in_maps = []
    for core in range(N_CORES):
        b, g = core // 2, core % 2
        u = idx[b]
        n = len(u)
        xk = np.zeros((Cp, D), dtype=np.float32)
        xk[:n] = x[b, u]
        keep = np.zeros(Cp, dtype=np.float32)
        keep[:n] = 1.0
        qs, ks, vs = g * FQ, D + g * FQ, 2 * D + g * FQ
        wqk = np.concatenate(
            [Wqkv[:, qs:qs + FQ], Wqkv[:, ks:ks + FQ]], axis=1
        )
        im = {
            "xT": np.ascontiguousarray(xk.T).astype(BF16),
            "wqk": np.ascontiguousarray(wqk).astype(BF16),
            "wv": np.ascontiguousarray(Wqkv[:, vs:vs + FQ]).astype(BF16),
            "wp": np.ascontiguousarray(Wproj[g * FQ:(g + 1) * FQ, :]).astype(BF16),
            "keep": np.ascontiguousarray(keep.reshape(NC, 128).T),
            "keepb": np.ascontiguousarray(keep.reshape(NC, 128).T).astype(BF16),
        }
        if with_bias:
            bqk = np.concatenate([bqkv[qs:qs + FQ], bqkv[ks:ks + FQ]])
            im["bqk"] = bqk.reshape(1, -1).astype(BF16)
            im["bv"] = bqkv[vs:vs + FQ].reshape(1, -1).astype(BF16)
            im["ones"] = np.ones((1, Cp), dtype=BF16)
        in_maps.append(im)

    global _last_in_maps
    _last_in_maps = in_maps
    res = run_bass_kernel_spmd(nc, in_maps, core_ids=list(range(N_CORES)))

    out = np.broadcast_to(bproj, (B, C, D)).copy()
    for b in range(B):
        u = idx[b]
        n = len(u)
        comb = res.results[2 * b]["outT"] + res.results[2 * b + 1]["outT"]
        out[b, u] += comb.T[:n]
    return out
